# revision 21
# baseline (speedup 1.0000x reference)
# Trainium2 Bass kernel for a Longformer-style sparse-attention encoder.
#
# Sharding: 8 NeuronCores = 2 (batch) x 4 (sequence shards of 1024 tokens).
# Per layer, within each 4-core group:
#   - AllGather #1a: the 3 global tokens' hidden rows (owners contribute).
#   - AllGather #1b: 128-token K/V halo boundaries for the sliding window.
#   - AllGather #2: partial softmax stats for the 3 global query rows
#     (distributed full attention, flash-style combine without max).
# Matmuls run in bf16 with fp32 PSUM accumulation; softmax, LayerNorm and the
# residual stream stay fp32. Activations are kept feature-major ("x^T") for
# matmul chains and token-major for LayerNorm/softmax-denominator steps.
# The final layer computes only the CLS-row path (the model output is the
# pooled CLS vector), skipping local attention and the non-CLS FFN entirely.
import os
import numpy as np
import ml_dtypes

import concourse.bass as bass
import concourse.tile as tile
from concourse import bacc, mybir
from concourse.bass_utils import run_bass_kernel_spmd
from concourse.masks import make_identity

F32 = mybir.dt.float32
BF16 = mybir.dt.bfloat16
FP16 = mybir.dt.float16
AF = mybir.ActivationFunctionType
ALU = mybir.AluOpType

B, S0, S1, HID, L, FF, NH, DH, WIN = 2, 2813, 1280, 768, 6, 3072, 12, 64, 128
S = 3 + S0 + S1          # 4096
SP, T = 4, 1024          # sequence shards, tokens per shard
NT, NQC, NF, NFM = 8, 4, 6, 24
G_IDX = (0, 1 + S0, S - 1)
LN_EPS = 1e-5
RG = [[0, 1, 2, 3], [4, 5, 6, 7]]
bfloat16 = ml_dtypes.bfloat16

KB_ELEMS = 2 * 128 * NF * 128      # k^T boundary slabs [side, p, f, n]
VB_ELEMS = 2 * 128 * HID           # v boundary slabs [side, p, n]
AG1B_ELEMS = KB_ELEMS + VB_ELEMS
AG2_COLS = HID + 32                # 768 partial out + stride-2 denoms + pad


def build(nlayers=6, dbg=False):
    STAGE = int(os.environ.get("KERN_STAGE", "9"))
    nc = bacc.Bacc("TRN2", target_bir_lowering=False, debug=False, num_devices=8)

    xin = nc.dram_tensor("xin", [T, HID], F32, kind="ExternalInput")
    emb = nc.dram_tensor("emb", [T, HID], F32, kind="ExternalInput")
    wq = nc.dram_tensor("wq", [L, 128, NF, HID], FP16, kind="ExternalInput")
    wk = nc.dram_tensor("wk", [L, 128, NF, HID], FP16, kind="ExternalInput")
    wv = nc.dram_tensor("wv", [L, 128, NF, HID], FP16, kind="ExternalInput")
    wkg = nc.dram_tensor("wkg", [L, 128, NF, HID], FP16, kind="ExternalInput")
    wvg = nc.dram_tensor("wvg", [L, 128, NF, HID], FP16, kind="ExternalInput")
    wqg = nc.dram_tensor("wqg", [L, 128, NF, HID], FP16, kind="ExternalInput")
    wo = nc.dram_tensor("wo", [L, 128, NF, HID], FP16, kind="ExternalInput")
    wi = nc.dram_tensor("wi", [L, 128, NF, FF], FP16, kind="ExternalInput")
    wff = nc.dram_tensor("wff", [L, 128, NFM, HID], FP16, kind="ExternalInput")
    wp = nc.dram_tensor("wp", [128, NF, HID], FP16, kind="ExternalInput")
    m_loc = nc.dram_tensor("m_loc", [NQC, 4, 128, 256], FP16, kind="ExternalInput")
    m_glb = nc.dram_tensor("m_glb", [NQC, 3, 256], FP16, kind="ExternalInput")
    selT = nc.dram_tensor("selT", [T, 3], F32, kind="ExternalInput")
    scat = nc.dram_tensor("scat", [3, T], FP16, kind="ExternalInput")
    oneh = nc.dram_tensor("oneh", [1, T], FP16, kind="ExternalInput")
    wsel = nc.dram_tensor("wsel", [1, 8], F32, kind="ExternalInput")

    pooled = nc.dram_tensor("pooled", [1, HID], F32, kind="ExternalOutput")
    dbg_attn = (nc.dram_tensor("dbg_attn", [128, NF, T], mybir.dt.float16,
                               kind="ExternalOutput")
                if int(os.environ.get("KERN_DBG_ATTN", "0")) else None)
    want_h = dbg or nlayers < L
    if want_h:
        hout = nc.dram_tensor("hout", [T, HID], F32, kind="ExternalOutput")

    with tile.TileContext(nc) as tc:
        import contextlib
        ctx = contextlib.ExitStack()
        with ctx:
            one = ctx.enter_context(tc.tile_pool(name="one", bufs=1))
            big = ctx.enter_context(tc.tile_pool(name="big", bufs=1))
            hpool = ctx.enter_context(tc.tile_pool(name="hpool", bufs=1))
            wpool = ctx.enter_context(tc.tile_pool(name="wpool", bufs=2))
            w3 = ctx.enter_context(tc.tile_pool(name="w3", bufs=2))
            w2 = ctx.enter_context(tc.tile_pool(name="w2", bufs=2))
            work1 = ctx.enter_context(tc.tile_pool(name="work1", bufs=1))
            tiny = ctx.enter_context(tc.tile_pool(name="tiny", bufs=1))
            dram = ctx.enter_context(tc.tile_pool(name="dram", bufs=2, space="DRAM"))
            psB = ctx.enter_context(tc.tile_pool(name="psB", bufs=3, space="PSUM"))
            psS = ctx.enter_context(tc.tile_pool(name="psS", bufs=3, space="PSUM"))
            psT = ctx.enter_context(tc.tile_pool(name="psT", bufs=2, space="PSUM"))

            ident = one.tile([128, 128], F32)
            make_identity(nc, ident)
            ident_b = one.tile([128, 128], FP16)
            make_identity(nc, ident_b)
            ones_bf = one.tile([128, 1], FP16)
            nc.vector.memset(ones_bf, 1.0)
            eps_t = one.tile([128, 1], F32)
            nc.vector.memset(eps_t, LN_EPS)
            ones_f = one.tile([1, 128], F32)
            nc.vector.memset(ones_f, 1.0)
            expb = one.tile([128, 1], F32)
            nc.vector.memset(expb, 0.0)

            m_loc_sb = one.tile([128, NQC, 4, 256], FP16)
            nc.sync.dma_start(m_loc_sb[:], m_loc.rearrange("q k p n -> p q k n"))
            m_glb_sb = one.tile([3, NQC, 256], FP16)
            nc.sync.dma_start(m_glb_sb[:], m_glb.rearrange("q g n -> g q n"))
            selT_sb = one.tile([128, NT, 3], F32)
            nc.sync.dma_start(selT_sb[:], selT.rearrange("(t p) g -> p t g", p=128))
            scat_sb = one.tile([3, T], FP16)
            nc.sync.dma_start(scat_sb[:], scat[:])
            oneh_sb = one.tile([1, T], FP16)
            nc.sync.dma_start(oneh_sb[:], oneh[:])
            wsel_sb = one.tile([128, 8], F32)
            nc.sync.dma_start(wsel_sb[:], wsel[0:1, :].to_broadcast([128, 8]))

            def ln(out_ap, in_ap, p=128):
                stats = w3.tile([p, 3, 6], F32, tag="ln_stats", name="ln_stats")
                inr = in_ap.rearrange("p (n f) -> p n f", f=256)
                for i in range(3):
                    nc.vector.bn_stats(out=stats[:, i, :], in_=inr[:, i, :])
                mv = w3.tile([p, 2], F32, tag="ln_mv", name="ln_mv")
                nc.vector.bn_aggr(out=mv[:], in_=stats[:])
                rstd = w3.tile([p, 1], F32, tag="ln_rstd", name="ln_rstd")
                nc.scalar.activation(out=rstd[:], in_=mv[:, 1:2], func=AF.Sqrt,
                                     bias=eps_t[:p])
                nc.vector.reciprocal(out=rstd[:], in_=rstd[:])
                nc.vector.tensor_scalar(out=out_ap, in0=in_ap, scalar1=mv[:, 0:1],
                                        scalar2=rstd[:], op0=ALU.subtract,
                                        op1=ALU.mult)

            # ---------------- embedding ----------------
            h = big.tile([128, NT, HID], F32, tag="h", name="h")
            xr = xin.rearrange("(t p) f -> p t f", p=128)
            er = emb.rearrange("(t p) f -> p t f", p=128)
            for tt in range(NT):
                xt = w3.tile([128, HID], F32, tag="tmp768", name="emb_x")
                et = w3.tile([128, HID], F32, tag="tmp768", name="emb_e")
                nc.sync.dma_start(xt[:], xr[:, tt])
                nc.sync.dma_start(et[:], er[:, tt])
                nc.vector.tensor_add(out=xt[:], in0=xt[:], in1=et[:])
                ln(h[:, tt, :], xt[:])

            def transpose_tm_to_fm(src_ap, dst):
                # src token-major [128, NT, 768] fp32 -> dst [128, NF, T] bf16
                for tt in range(NT):
                    cb = w3.tile([128, HID], FP16, tag="tmpb768", name="cb")
                    nc.any.tensor_copy(out=cb[:], in_=src_ap[:, tt, :])
                    for fi in range(NF):
                        pst = psT.tile([128, 512], FP16, tag="psT", name="pst")
                        nc.tensor.transpose(
                            pst[:, 0:128], cb[:, 128 * fi:128 * (fi + 1)],
                            ident_b[:])
                        nc.any.tensor_copy(
                            out=dst[:, fi, 128 * tt:128 * (tt + 1)],
                            in_=pst[:, 0:128])

            def load_w(wdram, lidx, cols=None, name="wt"):
                wt = wpool.tile([128, NF, HID], FP16, tag="wt", name=name)
                if cols is None:
                    nc.sync.dma_start(wt[:], wdram[lidx])
                else:
                    nc.sync.dma_start(wt[:], wdram[lidx, :, :, cols[0]:cols[1]])
                return wt

            def proj_fm(wt, hT, out):
                # out [128, NF, T] bf16 feature-major
                for fo in range(NF):
                    for c in range(T // 512):
                        psb = psB.tile([128, 512], F32, tag="psB", name="psb")
                        for fi in range(NF):
                            nc.tensor.matmul(
                                psb[:], wt[:, fi, 128 * fo:128 * (fo + 1)],
                                hT[:, fi, 512 * c:512 * (c + 1)],
                                start=(fi == 0), stop=(fi == NF - 1))
                        nc.any.tensor_copy(
                            out=out[:, fo, 512 * c:512 * (c + 1)], in_=psb[:])

            def proj_tm(wt, hT, out):
                # out [128, NT, 768] bf16 token-major
                for tt in range(NT):
                    for n0, nw in ((0, 512), (512, 256)):
                        psb = psB.tile([128, 512], F32, tag="psB", name="psb")
                        for fi in range(NF):
                            nc.tensor.matmul(
                                psb[:, :nw], hT[:, fi, 128 * tt:128 * (tt + 1)],
                                wt[:, fi, n0:n0 + nw],
                                start=(fi == 0), stop=(fi == NF - 1))
                        nc.any.tensor_copy(
                            out=out[:, tt, n0:n0 + nw], in_=psb[:, :nw])

            def small_proj(srcT, wt, name, odt=FP16):
                # srcT [128, NF, 3] -> [3, 768] bf16 token-major
                ps3a = psS.tile([128, 512], F32, tag="psS", name="ps3a")
                ps3b = psS.tile([128, 512], F32, tag="psS", name="ps3b")
                for pss, (n0, nw) in ((ps3a, (0, 512)), (ps3b, (512, 256))):
                    for fi in range(NF):
                        nc.tensor.matmul(pss[0:3, :nw], srcT[:, fi, :],
                                         wt[:, fi, n0:n0 + nw],
                                         start=(fi == 0), stop=(fi == NF - 1))
                ob = tiny.tile([3, HID], odt, tag=name, name=name)
                nc.any.tensor_copy(out=ob[:, 0:512], in_=ps3a[0:3, :512])
                nc.any.tensor_copy(out=ob[:, 512:768], in_=ps3b[0:3, :256])
                return ob

            def transpose3(src, name):
                # [3, 768] bf16 -> [128, NF, 3] bf16
                out = tiny.tile([128, NF, 3], FP16, tag=name + "T", name=name + "T")
                for fi in range(NF):
                    pst = psT.tile([128, 512], FP16, tag="psT", name="pst3")
                    nc.tensor.transpose(pst[:, 0:3], src[:, 128 * fi:128 * (fi + 1)],
                                        ident_b[0:3, 0:3])
                    nc.any.tensor_copy(out=out[:, fi, :], in_=pst[:, 0:3])
                return out

            hT = None
            for l in range(nlayers):
                last = (l == L - 1) and nlayers == L
                # -------- h^T --------
                hT = hpool.tile([128, NF, T], FP16, tag="hT", name="hT")
                transpose_tm_to_fm(h, hT)

                # -------- hg extraction + AG1a --------
                ps_hg1 = psS.tile([128, 512], F32, tag="psS", name="ps_hg1")
                ps_hg2 = psS.tile([128, 512], F32, tag="psS", name="ps_hg2")
                for pss, (n0, nw) in ((ps_hg1, (0, 512)), (ps_hg2, (512, 256))):
                    for tt in range(NT):
                        nc.tensor.matmul(pss[0:3, :nw], selT_sb[:, tt, :],
                                         h[:, tt, n0:n0 + nw],
                                         start=(tt == 0), stop=(tt == NT - 1))
                hg_cand = work1.tile([4, HID], F32, tag="slabx", name="hg_cand")
                nc.vector.memset(hg_cand[:], 0.0)
                nc.any.tensor_copy(out=hg_cand[0:3, 0:512], in_=ps_hg1[0:3, :512])
                nc.any.tensor_copy(out=hg_cand[0:3, 512:768], in_=ps_hg2[0:3, :256])
                ag1a_in = dram.tile([4, HID], F32, tag="ag1a_in", name="ag1a_in")
                ag1a_out = dram.tile([SP, 4, HID], F32, tag="ag1a_out",
                                     name="ag1a_out")
                nc.gpsimd.dma_start(ag1a_in[:], hg_cand[:])
                nc.gpsimd.collective_compute(
                    "AllGather", ALU.bypass, replica_groups=RG,
                    ins=[ag1a_in.opt()], outs=[ag1a_out.opt()])

                if not last:
                    # -------- k^T, v projections + AG1b --------
                    wt_k = load_w(wk, l, name="wt_k")
                    kT = hpool.tile([128, NF, T], FP16, tag="kT_mid", name="kT")
                    proj_fm(wt_k, hT, kT)
                    wt_v = load_w(wv, l, name="wt_v")
                    v_sb = hpool.tile([128, NT, HID], FP16, tag="v", name="v_sb")
                    proj_tm(wt_v, hT, v_sb)

                    ag1k_in = dram.tile([KB_ELEMS], FP16, tag="ag1k_in",
                                        name="ag1k_in")
                    ag1k_out = dram.tile([SP, KB_ELEMS], FP16, tag="ag1k_out",
                                         name="ag1k_out")
                    ag1v_in = dram.tile([VB_ELEMS], FP16, tag="ag1v_in",
                                        name="ag1v_in")
                    ag1v_out = dram.tile([SP, VB_ELEMS], FP16, tag="ag1v_out",
                                         name="ag1v_out")
                    kb = ag1k_in.rearrange("(s p f n) -> s p f n", s=2, p=128, f=NF)
                    vb = ag1v_in.rearrange("(s p n) -> s p n", s=2, p=128)
                    nc.gpsimd.dma_start(kb[0], kT[:, :, 0:128])
                    nc.gpsimd.dma_start(kb[1], kT[:, :, T - 128:T])
                    nc.gpsimd.dma_start(vb[0], v_sb[:, 0, :])
                    nc.gpsimd.dma_start(vb[1], v_sb[:, NT - 1, :])
                    nc.gpsimd.collective_compute(
                        "AllGather", ALU.bypass, replica_groups=RG,
                        ins=[ag1k_in.opt()], outs=[ag1k_out.opt()])
                    nc.gpsimd.collective_compute(
                        "AllGather", ALU.bypass, replica_groups=RG,
                        ins=[ag1v_in.opt()], outs=[ag1v_out.opt()])

                # -------- kgf^T, vgf --------
                wt_kg = load_w(wkg, l, name="wt_kg")
                kgfT = hpool.tile([128, NF, T], FP16, tag="kgfT_attnT", name="kgfT")
                proj_fm(wt_kg, hT, kgfT)
                wt_vg = load_w(wvg, l, name="wt_vg")
                vgf = hpool.tile([128, NT, HID], FP16, tag="qT_vgf", name="vgf")
                proj_tm(wt_vg, hT, vgf)

                if STAGE < 2:
                    for tt in range(NT):
                        dmy = w3.tile([128, HID], F32, tag="tmp768", name="dmy")
                        nc.vector.tensor_copy(out=dmy[:], in_=h[:, tt, :])
                        ln(h[:, tt, :], dmy[:])
                    continue
                # -------- combine hg; qg/kg/vg --------
                hg_f = work1.tile([3, HID], F32, tag="hgf", name="hg_f")
                nc.sync.dma_start(hg_f[:], ag1a_out[0, 0:3])
                for j in range(1, SP):
                    hgs = w2.tile([3, AG2_COLS], F32, tag="agld", name="hgs")
                    nc.sync.dma_start(hgs[:, 0:HID], ag1a_out[j, 0:3])
                    nc.vector.tensor_add(out=hg_f[:], in0=hg_f[:],
                                         in1=hgs[:, 0:HID])
                hg = tiny.tile([3, HID], FP16, tag="qgkg", name="hg")
                nc.vector.tensor_copy(out=hg[:], in_=hg_f[:])
                hgT = transpose3(hg, "hg")
                wt_qg = load_w(wqg, l, name="wt_qg")
                qg = small_proj(hgT, wt_qg, "qgkg")
                qgT = transpose3(qg, "qg")
                if not last:
                    wt_kg2 = load_w(wk, l, name="wt_kg2")
                    kg = small_proj(hgT, wt_kg2, "qgkg")
                    kgT = transpose3(kg, "kg")
                    wt_vg2 = load_w(wv, l, name="wt_vg2")
                    vg = small_proj(hgT, wt_vg2, "vg")

                # -------- global-row partial attention + AG2 --------
                # heads split even/odd across two psum tiles: concurrent
                # row-group matmuls must not share a PSUM bank.
                Eg2 = w2.tile([128, NT, 48], FP16, tag="Eg2", name="Eg2")
                for tt in range(NT):
                    ps_sgA = psS.tile([128, 512], F32, tag="psS", name="ps_sgA")
                    ps_sgB = psS.tile([128, 512], F32, tag="psS", name="ps_sgB")
                    nc.vector.memset(ps_sgA[:, 0:24], 0.0)
                    nc.vector.memset(ps_sgB[:, 0:24], 0.0)
                    for hh in range(NH):
                        pt = ps_sgA if hh % 2 == 0 else ps_sgB
                        nc.tensor.matmul(
                            pt[0:128, 4 * (hh // 2):4 * (hh // 2) + 3],
                            kgfT[64 * (hh % 2):64 * (hh % 2) + 64, hh // 2,
                                 128 * tt:128 * (tt + 1)],
                            qgT[64 * (hh % 2):64 * (hh % 2) + 64, hh // 2, :],
                            start=True, stop=True)
                    nc.scalar.activation(out=Eg2[:, tt, 0:24], in_=ps_sgA[:, 0:24],
                                         func=AF.Exp, bias=expb[:])
                    nc.scalar.activation(out=Eg2[:, tt, 24:48], in_=ps_sgB[:, 0:24],
                                         func=AF.Exp, bias=expb[:])
                ps_po1 = psS.tile([128, 512], F32, tag="psS", name="ps_po1")
                ps_po2 = psS.tile([128, 512], F32, tag="psS", name="ps_po2")
                ps_pdt = psB.tile([128, 512], F32, tag="psB", name="ps_pdt")
                ps_pd = ps_pdt[:, 0:24]
                nc.vector.memset(ps_pd[0:3, :], 0.0)
                for hh in range(NH):
                    po, n0 = (ps_po1, 0) if hh < 8 else (ps_po2, 512)
                    for tt in range(NT):
                        c0 = 24 * (hh % 2) + 4 * (hh // 2)
                        nc.tensor.matmul(ps_pd[0:3, 2 * hh:2 * hh + 1],
                                         Eg2[:, tt, c0:c0 + 3],
                                         ones_bf[:, :],
                                         start=(tt == 0), stop=(tt == NT - 1))
                        nc.tensor.matmul(
                            po[0:3, 64 * hh - n0:64 * hh - n0 + 64],
                            Eg2[:, tt, c0:c0 + 3],
                            vgf[:, tt, 64 * hh:64 * hh + 64],
                            start=(tt == 0), stop=(tt == NT - 1))
                ag2_in = dram.tile([3, AG2_COLS], F32, tag="ag2_in", name="ag2_in")
                ag2_out = dram.tile([SP, 3, AG2_COLS], F32, tag="ag2_out",
                                    name="ag2_out")
                slab = work1.tile([3, AG2_COLS], F32, tag="slabx", name="slab")
                nc.any.tensor_copy(out=slab[:, 0:512], in_=ps_po1[0:3, :512])
                nc.any.tensor_copy(out=slab[:, 512:768], in_=ps_po2[0:3, :256])
                nc.any.tensor_copy(out=slab[:, 768:792], in_=ps_pd[0:3, :])
                nc.vector.memset(slab[:, 792:], 0.0)
                nc.gpsimd.dma_start(ag2_in[:], slab[:])
                nc.gpsimd.collective_compute(
                    "AllGather", ALU.bypass, replica_groups=RG,
                    ins=[ag2_in.opt()], outs=[ag2_out.opt()])

                # -------- AG2 combine -> cmb --------
                psum_ = work1.tile([3, AG2_COLS], F32, tag="psumx", name="psum_")
                nc.sync.dma_start(psum_[:], ag2_out[0])
                for j in range(1, SP):
                    a2s = w2.tile([3, AG2_COLS], F32, tag="agld", name="a2s")
                    nc.sync.dma_start(a2s[:], ag2_out[j])
                    nc.vector.tensor_add(out=psum_[:], in0=psum_[:], in1=a2s[:])
                rd = tiny.tile([3, 24], F32, tag="rd", name="rd")
                nc.vector.reciprocal(out=rd[:], in_=psum_[:, 768:792])
                cmb_f = work1.tile([3, HID], F32, tag="cmb_f", name="cmb_f")
                for hh in range(NH):
                    nc.vector.tensor_scalar_mul(
                        out=cmb_f[:, 64 * hh:64 * hh + 64],
                        in0=psum_[:, 64 * hh:64 * hh + 64],
                        scalar1=rd[:, 2 * hh:2 * hh + 1])
                cmb = tiny.tile([3, HID], FP16, tag="cmb", name="cmb")
                nc.vector.tensor_copy(out=cmb[:], in_=cmb_f[:])

                if last:
                    # -------- final layer: CLS-only path --------
                    cls_o = work1.tile([1, HID], F32, tag="clsA", name="cls_o")
                    nc.vector.tensor_copy(out=cls_o[:], in_=cmb_f[0:1, :])
                    cls_b = tiny.tile([1, HID], FP16, tag="clsb", name="cls_b")
                    nc.vector.tensor_copy(out=cls_b[:], in_=cls_o[:])

                    def transpose1(src_b, nsub, name):
                        out = tiny.tile([128, nsub, 1], FP16, tag=name + "T1",
                                        name=name + "T1")
                        for fi in range(nsub):
                            pst = psT.tile([128, 512], FP16, tag="psT",
                                           name="pst1")
                            nc.tensor.transpose(
                                pst[:, 0:1], src_b[:, 128 * fi:128 * (fi + 1)],
                                ident_b[0:1, 0:1])
                            nc.any.tensor_copy(out=out[:, fi, :], in_=pst[:, 0:1])
                        return out

                    def rowmm(srcT, wdram, lidx, nsub, name, tag):
                        # [1, 768] = srcT . W  (W streamed in [*,6,768] chunks)
                        ps_a = psS.tile([128, 512], F32, tag="psS", name="ps_ra")
                        ps_b = psS.tile([128, 512], F32, tag="psS", name="ps_rb")
                        nchunk = (nsub + NF - 1) // NF
                        for ch in range(nchunk):
                            wt_c = wpool.tile([128, NF, HID], FP16, tag="wt",
                                              name="wt_row")
                            nc.sync.dma_start(
                                wt_c[:], wdram[lidx, :, NF * ch:NF * (ch + 1), :]
                                if lidx is not None else
                                wdram[:, NF * ch:NF * (ch + 1), :])
                            for pss, (n0, nw) in ((ps_a, (0, 512)),
                                                  (ps_b, (512, 256))):
                                for fi in range(NF):
                                    k = NF * ch + fi
                                    nc.tensor.matmul(
                                        pss[0:1, :nw], srcT[:, k, :],
                                        wt_c[:, fi, n0:n0 + nw],
                                        start=(k == 0), stop=(k == nsub - 1))
                        ro = work1.tile([1, HID], F32, tag=tag, name=name)
                        nc.any.tensor_copy(out=ro[:, 0:512], in_=ps_a[0:1, :512])
                        nc.any.tensor_copy(out=ro[:, 512:768], in_=ps_b[0:1, :256])
                        return ro

                    clsT = transpose1(cls_b, NF, "cls")
                    att_o = rowmm(clsT, wo, l, NF, "att_o", "clsB")
                    nc.vector.tensor_add(out=att_o[:], in0=att_o[:],
                                         in1=h[0:1, 0, :])
                    h1c = work1.tile([1, HID], F32, tag="clsA", name="h1c")
                    ln(h1c[:], att_o[:], p=1)
                    h1cb = tiny.tile([1, HID], FP16, tag="clsb", name="h1cb")
                    nc.vector.tensor_copy(out=h1cb[:], in_=h1c[:])
                    h1cT = transpose1(h1cb, NF, "h1c")
                    midc = work1.tile([1, FF], FP16, tag="slabx", name="midc")
                    for ch in range(4):
                        wt_c = wpool.tile([128, NF, HID], FP16, tag="wt",
                                          name="wt_ffn1")
                        nc.sync.dma_start(wt_c[:],
                                          wi[l, :, :, 768 * ch:768 * (ch + 1)])
                        for c2 in range(2):
                            n0 = 768 * ch + 512 * c2
                            nw = 512 if c2 == 0 else 256
                            psf = psS.tile([128, 512], F32, tag="psS", name="psf")
                            for fi in range(NF):
                                nc.tensor.matmul(
                                    psf[0:1, :nw], h1cT[:, fi, :],
                                    wt_c[:, fi, 512 * c2:512 * c2 + nw],
                                    start=(fi == 0), stop=(fi == NF - 1))
                            nc.scalar.activation(out=midc[:, n0:n0 + nw],
                                                 in_=psf[0:1, :nw], func=AF.Gelu)
                    midcT = transpose1(midc, NFM, "midc")
                    ff_o = rowmm(midcT, wff, l, NFM, "ff_o", "clsC")
                    nc.vector.tensor_add(out=ff_o[:], in0=ff_o[:], in1=h1c[:])
                    h2c = work1.tile([1, HID], F32, tag="clsB", name="h2c")
                    ln(h2c[:], ff_o[:], p=1)
                    h2cb = tiny.tile([1, HID], FP16, tag="clsb", name="h2cb")
                    nc.vector.tensor_copy(out=h2cb[:], in_=h2c[:])
                    h2cT = transpose1(h2cb, NF, "h2c")
                    pl = rowmm(h2cT, wp, None, NF, "pl", "clsA")
                    plt = work1.tile([1, HID], F32, tag="clsC", name="plt")
                    nc.scalar.activation(out=plt[:], in_=pl[:], func=AF.Tanh)
                    nc.sync.dma_start(pooled[:], plt[:])
                    break

                if STAGE < 3:
                    for tt in range(NT):
                        dmy = w3.tile([128, HID], F32, tag="tmp768", name="dmy")
                        nc.vector.tensor_copy(out=dmy[:], in_=h[:, tt, :])
                        ln(h[:, tt, :], dmy[:])
                    continue
                # -------- q^T --------
                wt_q = load_w(wq, l, name="wt_q")
                qT = hpool.tile([128, NF, T], FP16, tag="qT_vgf", name="qT")
                proj_fm(wt_q, hT, qT)

                # -------- halo select --------
                klT = work1.tile([128, NF, 128], FP16, tag="klT", name="klT")
                krT = work1.tile([128, NF, 128], FP16, tag="krT", name="krT")
                vl = work1.tile([128, HID], FP16, tag="vl", name="vl")
                vr = work1.tile([128, HID], FP16, tag="vr", name="vr")
                kbo = ag1k_out.rearrange(
                    "s (b p f n) -> s b p f n", b=2, p=128, f=NF)
                vbo = ag1v_out.rearrange(
                    "s (b p n) -> s b p n", b=2, p=128)
                for dst, src_of, side, wofs in (
                        (klT, kbo, 1, 0), (krT, kbo, 0, 4),
                        (vl, vbo, 1, 0), (vr, vbo, 0, 4)):
                    shp = [128, NF, 128] if dst in (klT, krT) else [128, HID]
                    sdt = FP16
                    for j in range(SP):
                        sl = w3.tile(shp, sdt, tag="halo_sl", name="halo_sl")
                        nc.sync.dma_start(sl[:], src_of[j, side])
                        if j == 0:
                            nc.vector.tensor_scalar_mul(
                                out=dst[:], in0=sl[:],
                                scalar1=wsel_sb[:, wofs + j:wofs + j + 1])
                        else:
                            tmp = w3.tile(shp, sdt, tag="halo_sl",
                                          name="halo_tmp")
                            nc.vector.tensor_scalar_mul(
                                out=tmp[:], in0=sl[:],
                                scalar1=wsel_sb[:, wofs + j:wofs + j + 1])
                            nc.vector.tensor_add(out=dst[:], in0=dst[:],
                                                 in1=tmp[:])

                # -------- local attention --------
                attnT = hpool.tile([128, NF, T], FP16, tag="kgfT_attnT",
                                   name="attnT")

                def k_slice(ktl, hh):
                    p0 = 64 * (hh % 2)
                    if ktl < 0:
                        return klT[p0:p0 + 64, hh // 2, :]
                    if ktl >= NT:
                        return krT[p0:p0 + 64, hh // 2, :]
                    return kT[p0:p0 + 64, hh // 2, 128 * ktl:128 * (ktl + 1)]

                def v_slice(ktl, hh):
                    if ktl < 0:
                        return vl[:, 64 * hh:64 * hh + 64]
                    if ktl >= NT:
                        return vr[:, 64 * hh:64 * hh + 64]
                    return v_sb[:, ktl, 64 * hh:64 * hh + 64]

                for qc in (1, 2, 0, 3):
                    q0 = 256 * qc
                    kts = [2 * qc - 1, 2 * qc, 2 * qc + 1, 2 * qc + 2]
                    for hp in range(NH // 2):
                        dbc_ps = psB.tile([128, 512], F32, tag="psB",
                                          name="dbc_ps")
                        ps_pv = psB.tile([128, 512], F32, tag="psB", name="ps_pv")
                        for m in range(2):
                            hh = 2 * hp + m
                            p0 = 64 * m
                            E = w2.tile([128, 4, 256], FP16, tag="E", name="E")
                            ps_den = psS.tile([128, 512], F32, tag="psS",
                                              name="ps_den")
                            for j, ktl in enumerate(kts):
                                ps_s = psB.tile([128, 512], F32, tag="psB",
                                                name="ps_s")
                                nc.tensor.matmul(
                                    ps_s[:, 0:256], k_slice(ktl, hh),
                                    qT[p0:p0 + 64, hp, q0:q0 + 256],
                                    start=True, stop=True)
                                nc.scalar.activation(out=E[:, j, :],
                                                     in_=ps_s[:, 0:256],
                                                     func=AF.Exp, bias=expb[:])
                                eng = nc.vector if (j + hh) % 2 else nc.gpsimd
                                eng.tensor_tensor(
                                    out=E[:, j, :], in0=E[:, j, :],
                                    in1=m_loc_sb[:, qc, j, :], op=ALU.mult)
                            ps_sg2 = psS.tile([128, 512], F32, tag="psS",
                                              name="ps_sg2")
                            nc.tensor.matmul(ps_sg2[0:3, 0:256],
                                             kgT[p0:p0 + 64, hp, :],
                                             qT[p0:p0 + 64, hp, q0:q0 + 256],
                                             start=True, stop=True)
                            Eg = w2.tile([3, 256], FP16, tag="Egl", name="Egl")
                            nc.scalar.activation(out=Eg[:], in_=ps_sg2[0:3, 0:256],
                                                 func=AF.Exp, bias=expb[0:3])
                            nc.vector.tensor_tensor(out=Eg[:], in0=Eg[:],
                                                    in1=m_glb_sb[:, qc, :],
                                                    op=ALU.mult)
                            for j in range(4):
                                nc.tensor.matmul(ps_den[0:1, 0:256],
                                                 ones_bf[:, :], E[:, j, :],
                                                 start=(j == 0), stop=False)
                            nc.tensor.matmul(ps_den[0:1, 0:256], ones_bf[0:3, :],
                                             Eg[:], start=False, stop=False)
                            nc.tensor.matmul(ps_den[0:1, 0:256], ones_bf[0:1, :],
                                             oneh_sb[:, q0:q0 + 256],
                                             start=False, stop=True)
                            den = w3.tile([1, 256], F32, tag="den", name="den")
                            nc.vector.reciprocal(out=den[:], in_=ps_den[0:1, 0:256])
                            nc.tensor.matmul(dbc_ps[p0:p0 + 64, 0:256],
                                             ones_f[:, 0:64], den[:],
                                             start=True, stop=True)
                            for j, ktl in enumerate(kts):
                                nc.tensor.matmul(ps_pv[p0:p0 + 64, 0:256],
                                                 v_slice(ktl, hh), E[:, j, :],
                                                 start=(j == 0), stop=False)
                            nc.tensor.matmul(ps_pv[p0:p0 + 64, 0:256],
                                             vg[:, 64 * hh:64 * hh + 64], Eg[:],
                                             start=False, stop=False)
                            nc.tensor.matmul(ps_pv[p0:p0 + 64, 0:256],
                                             cmb[:, 64 * hh:64 * hh + 64],
                                             scat_sb[:, q0:q0 + 256],
                                             start=False, stop=True)
                        dbc = w2.tile([128, 256], F32, tag="dbc", name="dbc")
                        nc.vector.tensor_copy(out=dbc[:], in_=dbc_ps[:, 0:256])
                        nc.vector.tensor_tensor(out=attnT[:, hp, q0:q0 + 256],
                                                in0=ps_pv[:, 0:256], in1=dbc[:],
                                                op=ALU.mult)

                if STAGE < 4:
                    for tt in range(NT):
                        dmy = w3.tile([128, HID], F32, tag="tmp768", name="dmy")
                        nc.vector.tensor_copy(out=dmy[:], in_=h[:, tt, :])
                        ln(h[:, tt, :], dmy[:])
                    continue
                if dbg_attn is not None and l == 0:
                    nc.sync.dma_start(dbg_attn[:], attnT[:])
                # -------- O proj + residual + LN1 --------
                wt_o = load_w(wo, l, name="wt_o")
                h1 = big.tile([128, NT, HID], F32, tag="h1", name="h1")
                for tt in range(NT):
                    hro = w3.tile([128, HID], F32, tag="tmp768", name="hro")
                    for n0, nw in ((0, 512), (512, 256)):
                        psb = psB.tile([128, 512], F32, tag="psB", name="psb")
                        for fi in range(NF):
                            nc.tensor.matmul(
                                psb[:, :nw], attnT[:, fi, 128 * tt:128 * (tt + 1)],
                                wt_o[:, fi, n0:n0 + nw],
                                start=(fi == 0), stop=(fi == NF - 1))
                        nc.vector.tensor_add(out=hro[:, n0:n0 + nw],
                                             in0=psb[:, :nw],
                                             in1=h[:, tt, n0:n0 + nw])
                    ln(h1[:, tt, :], hro[:])

                # -------- FFN --------
                h1T = hpool.tile([128, NF, T], FP16, tag="hT", name="h1T")
                transpose_tm_to_fm(h1, h1T)
                h_new = big.tile([128, NT, HID], F32, tag="h", name="h_new")
                for quarter in range(4):
                    t0 = 256 * quarter
                    midT = hpool.tile([128, NFM, 256], FP16, tag="kT_mid",
                                      name="midT")
                    for ch in range(4):
                        wt_c = wpool.tile([128, NF, HID], FP16, tag="wt",
                                          name="wt_i")
                        nc.sync.dma_start(wt_c[:],
                                          wi[l, :, :, 768 * ch:768 * (ch + 1)])
                        for fo in range(NF):
                            psb = psB.tile([128, 512], F32, tag="psB", name="psb")
                            for fi in range(NF):
                                nc.tensor.matmul(
                                    psb[:, 0:256],
                                    wt_c[:, fi, 128 * fo:128 * (fo + 1)],
                                    h1T[:, fi, t0:t0 + 256],
                                    start=(fi == 0), stop=(fi == NF - 1))
                            nc.scalar.activation(out=midT[:, NF * ch + fo, :],
                                                 in_=psb[:, 0:256], func=AF.Gelu)
                    ps_out = [
                        psB.tile([128, 512], F32, tag="psB", name="ps_out0"),
                        psB.tile([128, 512], F32, tag="psB", name="ps_out1"),
                        psS.tile([128, 512], F32, tag="psS", name="ps_out2"),
                        psS.tile([128, 512], F32, tag="psS", name="ps_out3"),
                    ]
                    for ch in range(4):
                        wt_c = wpool.tile([128, NF, HID], FP16, tag="wt",
                                          name="wt_f")
                        nc.sync.dma_start(
                            wt_c[:], wff[l, :, NF * ch:NF * (ch + 1), :])
                        for t2 in range(2):
                            for ci, (n0, nw) in enumerate(((0, 512),
                                                          (512, 256))):
                                for fi in range(NF):
                                    k = NF * ch + fi
                                    nc.tensor.matmul(
                                        ps_out[2 * t2 + ci][:, :nw],
                                        midT[:, k, 128 * t2:128 * (t2 + 1)],
                                        wt_c[:, fi, n0:n0 + nw],
                                        start=(k == 0), stop=(k == NFM - 1))
                    for t2 in range(2):
                        tt = 2 * quarter + t2
                        ffo = w3.tile([128, HID], F32, tag="tmp768",
                                      name="ffo")
                        for ci, (n0, nw) in enumerate(((0, 512), (512, 256))):
                            nc.vector.tensor_add(
                                out=ffo[:, n0:n0 + nw],
                                in0=ps_out[2 * t2 + ci][:, :nw],
                                in1=h1[:, tt, n0:n0 + nw])
                        ln(h_new[:, tt, :], ffo[:])
                h = h_new

            if want_h:
                nc.sync.dma_start(hout.rearrange("(t p) f -> p t f", p=128), h[:])
            if nlayers < L:
                zz = one.tile([1, HID], F32)
                nc.vector.memset(zz, 0.0)
                nc.sync.dma_start(pooled[:], zz[:])

    nc.compile()
    return nc


# ---------------- host side ----------------

def _host_prep(inputs):
    bf = lambda a: np.ascontiguousarray(a).astype(bfloat16)
    f16 = lambda a: np.ascontiguousarray(a).astype(np.float16)

    def tile_w(w, scale=1.0):
        w = np.asarray(w, np.float32) * scale
        Lx, fin, fout = w.shape
        return f16(w.reshape(Lx, fin // 128, 128, fout).transpose(0, 2, 1, 3))

    x = np.concatenate([
        np.broadcast_to(np.asarray(inputs["cls_tok"], np.float32)[None],
                        (B, 1, HID)),
        np.asarray(inputs["x0"], np.float32),
        np.broadcast_to(np.asarray(inputs["sep_tok"], np.float32)[None],
                        (B, 1, HID)),
        np.asarray(inputs["x1"], np.float32),
        np.broadcast_to(np.asarray(inputs["sep_tok"], np.float32)[None],
                        (B, 1, HID)),
    ], axis=1)
    qlen = S1 + 1
    tt_types = np.concatenate([np.zeros(S - qlen, np.int32),
                               np.ones(qlen, np.int32)])
    embf = (np.asarray(inputs["pos_emb"], np.float32)[np.arange(S) + 2]
            + np.asarray(inputs["type_emb"], np.float32)[tt_types])

    for k in ("bq", "bk", "bv", "bqg", "bkg", "bvg", "bo", "bi", "bff", "bp",
              "emb_ln_b", "ln1_b", "ln2_b"):
        assert not np.any(np.asarray(inputs[k])), f"nonzero bias {k} unsupported"
    for k in ("emb_ln_w", "ln1_w", "ln2_w"):
        assert np.all(np.asarray(inputs[k]) == 1.0), f"non-unit {k} unsupported"

    scale = 1.0 / np.sqrt(DH)
    weights = {
        "wq": tile_w(inputs["Wq"], scale), "wk": tile_w(inputs["Wk"]),
        "wv": tile_w(inputs["Wv"]), "wkg": tile_w(inputs["Wkg"]),
        "wvg": tile_w(inputs["Wvg"]), "wqg": tile_w(inputs["Wqg"], scale),
        "wo": tile_w(inputs["Wo"]), "wi": tile_w(inputs["Wi"]),
        "wff": tile_w(inputs["Wff"]),
        "wp": tile_w(np.asarray(inputs["Wp"])[None])[0],
    }

    per_r = []
    g = np.array(G_IDX)
    for r in range(SP):
        t_q = r * T + np.arange(T)
        m_loc = np.zeros((NQC, 4, 128, 256), np.float32)
        for qc in range(NQC):
            tq = t_q[256 * qc:256 * (qc + 1)]
            for j in range(4):
                ktl = 2 * qc - 1 + j
                tk = r * T + ktl * 128 + np.arange(128)
                valid = (np.abs(tk[:, None] - tq[None, :]) <= WIN)
                valid &= (tk[:, None] >= 0) & (tk[:, None] < S)
                valid &= ~np.isin(tk, g)[:, None]
                valid &= ~np.isin(tq, g)[None, :]
                m_loc[qc, j] = valid
        m_glb = np.broadcast_to(~np.isin(t_q, g), (3, T)).reshape(
            3, NQC, 256).transpose(1, 0, 2).astype(np.float32)
        oneh = np.isin(t_q, g).astype(np.float32)[None]
        selTm = np.zeros((T, 3), np.float32)
        scatm = np.zeros((3, T), np.float32)
        for gi, gp in enumerate(G_IDX):
            if r * T <= gp < (r + 1) * T:
                selTm[gp - r * T, gi] = 1.0
                scatm[gi, gp - r * T] = 1.0
        wselm = np.zeros((1, 8), np.float32)
        if r > 0:
            wselm[0, r - 1] = 1.0
        if r < SP - 1:
            wselm[0, 4 + r + 1] = 1.0
        per_r.append(dict(m_loc=f16(m_loc), m_glb=f16(np.ascontiguousarray(m_glb)),
                          oneh=f16(oneh), selT=selTm, scat=f16(scatm), wsel=wselm))

    in_maps = []
    for c in range(8):
        b, r = c // SP, c % SP
        m = dict(weights)
        m.update(per_r[r])
        m["xin"] = np.ascontiguousarray(x[b, r * T:(r + 1) * T])
        m["emb"] = np.ascontiguousarray(embf[r * T:(r + 1) * T])
        in_maps.append(m)
    return in_maps


_CACHE = {}


def _get_program(nlayers=6, dbg=False):
    key = (nlayers, dbg)
    if key not in _CACHE:
        _CACHE[key] = build(nlayers, dbg)
    return _CACHE[key]


def kernel(**inputs):
    nlayers = int(os.environ.get("KERN_NLAYERS", "6"))
    dbg = bool(int(os.environ.get("KERN_DBG", "0")))
    nc = _get_program(nlayers, dbg)
    in_maps = _host_prep(inputs)
    trace = bool(int(os.environ.get("KERN_TRACE", "0")))
    res = run_bass_kernel_spmd(nc, in_maps, core_ids=list(range(8)), trace=trace)
    kernel.last_results = res
    out = np.stack([res.results[0]["pooled"], res.results[4]["pooled"]])
    return out.astype(np.float32)


# revision 22
# speedup vs baseline: 1.2836x; 1.2836x over previous
# Trainium2 Bass kernel for a Longformer-style sparse-attention encoder.
#
# Sharding: 8 NeuronCores = 2 (batch) x 4 (sequence shards of 1024 tokens).
# Per layer, within each 4-core group:
#   - AllGather #1a: the 3 global tokens' hidden rows (owners contribute).
#   - AllGather #1b: 128-token K/V halo boundaries for the sliding window.
#   - AllGather #2: partial softmax stats for the 3 global query rows
#     (distributed full attention, flash-style combine without max).
# Matmuls run in bf16 with fp32 PSUM accumulation; softmax, LayerNorm and the
# residual stream stay fp32. Activations are kept feature-major ("x^T") for
# matmul chains and token-major for LayerNorm/softmax-denominator steps.
# The final layer computes only the CLS-row path (the model output is the
# pooled CLS vector), skipping local attention and the non-CLS FFN entirely.
import os
import numpy as np
import ml_dtypes

import concourse.bass as bass
import concourse.tile as tile
from concourse import bacc, mybir
from concourse.bass_utils import run_bass_kernel_spmd
from concourse.masks import make_identity

F32 = mybir.dt.float32
BF16 = mybir.dt.bfloat16
FP16 = mybir.dt.float16
AF = mybir.ActivationFunctionType
ALU = mybir.AluOpType

B, S0, S1, HID, L, FF, NH, DH, WIN = 2, 2813, 1280, 768, 6, 3072, 12, 64, 128
S = 3 + S0 + S1          # 4096
SP, T = 4, 1024          # sequence shards, tokens per shard
NT, NQC, NF, NFM = 8, 4, 6, 24
G_IDX = (0, 1 + S0, S - 1)
LN_EPS = 1e-5
RG = [[0, 1, 2, 3], [4, 5, 6, 7]]
bfloat16 = ml_dtypes.bfloat16

KB_ELEMS = 2 * 128 * NF * 128      # k^T boundary slabs [side, p, f, n]
VB_ELEMS = 2 * 128 * HID           # v boundary slabs [side, p, n]
AG1B_ELEMS = KB_ELEMS + VB_ELEMS
AG2_COLS = HID + 32                # 768 partial out + stride-2 denoms + pad


def build(nlayers=6, dbg=False):
    STAGE = int(os.environ.get("KERN_STAGE", "9"))
    nc = bacc.Bacc("TRN2", target_bir_lowering=False, debug=False, num_devices=8)

    xin = nc.dram_tensor("xin", [T, HID], F32, kind="ExternalInput")
    emb = nc.dram_tensor("emb", [T, HID], F32, kind="ExternalInput")
    wq = nc.dram_tensor("wq", [L, 128, NF, HID], FP16, kind="ExternalInput")
    wk = nc.dram_tensor("wk", [L, 128, NF, HID], FP16, kind="ExternalInput")
    wv = nc.dram_tensor("wv", [L, 128, NF, HID], FP16, kind="ExternalInput")
    wkg = nc.dram_tensor("wkg", [L, 128, NF, HID], FP16, kind="ExternalInput")
    wvg = nc.dram_tensor("wvg", [L, 128, NF, HID], FP16, kind="ExternalInput")
    wqg = nc.dram_tensor("wqg", [L, 128, NF, HID], FP16, kind="ExternalInput")
    wo = nc.dram_tensor("wo", [L, 128, NF, HID], FP16, kind="ExternalInput")
    wi = nc.dram_tensor("wi", [L, 128, NF, FF], FP16, kind="ExternalInput")
    wff = nc.dram_tensor("wff", [L, 128, NFM, HID], FP16, kind="ExternalInput")
    wp = nc.dram_tensor("wp", [128, NF, HID], FP16, kind="ExternalInput")
    m_loc = nc.dram_tensor("m_loc", [NQC, 4, 128, 256], FP16, kind="ExternalInput")
    m_glb = nc.dram_tensor("m_glb", [NQC, 3, 256], FP16, kind="ExternalInput")
    selT = nc.dram_tensor("selT", [T, 3], F32, kind="ExternalInput")
    scat = nc.dram_tensor("scat", [3, T], FP16, kind="ExternalInput")
    oneh = nc.dram_tensor("oneh", [1, T], FP16, kind="ExternalInput")
    wsel = nc.dram_tensor("wsel", [1, 8], F32, kind="ExternalInput")

    pooled = nc.dram_tensor("pooled", [1, HID], F32, kind="ExternalOutput")
    dbg_attn = (nc.dram_tensor("dbg_attn", [128, NF, T], mybir.dt.float16,
                               kind="ExternalOutput")
                if int(os.environ.get("KERN_DBG_ATTN", "0")) else None)
    want_h = dbg or nlayers < L
    if want_h:
        hout = nc.dram_tensor("hout", [T, HID], F32, kind="ExternalOutput")

    with tile.TileContext(nc) as tc:
        import contextlib
        ctx = contextlib.ExitStack()
        with ctx:
            one = ctx.enter_context(tc.tile_pool(name="one", bufs=1))
            big = ctx.enter_context(tc.tile_pool(name="big", bufs=1))
            hpool = ctx.enter_context(tc.tile_pool(name="hpool", bufs=1))
            wpool = ctx.enter_context(tc.tile_pool(name="wpool", bufs=2))
            w3 = ctx.enter_context(tc.tile_pool(name="w3", bufs=2))
            w2 = ctx.enter_context(tc.tile_pool(name="w2", bufs=2))
            work1 = ctx.enter_context(tc.tile_pool(name="work1", bufs=1))
            tiny = ctx.enter_context(tc.tile_pool(name="tiny", bufs=1))
            dram = ctx.enter_context(tc.tile_pool(name="dram", bufs=2, space="DRAM"))
            psB = ctx.enter_context(tc.tile_pool(name="psB", bufs=4, space="PSUM"))
            psS = ctx.enter_context(tc.tile_pool(name="psS", bufs=3, space="PSUM"))
            psT = ctx.enter_context(tc.tile_pool(name="psT", bufs=1, space="PSUM"))

            ident = one.tile([128, 128], F32)
            make_identity(nc, ident)
            ident_b = one.tile([128, 128], FP16)
            make_identity(nc, ident_b)
            ones_bf = one.tile([128, 1], FP16)
            nc.vector.memset(ones_bf, 1.0)
            eps_t = one.tile([128, 1], F32)
            nc.vector.memset(eps_t, LN_EPS)
            ones_f = one.tile([1, 128], F32)
            nc.vector.memset(ones_f, 1.0)
            expb = one.tile([128, 1], F32)
            nc.vector.memset(expb, 0.0)

            m_loc_sb = one.tile([128, NQC, 4, 256], FP16)
            nc.sync.dma_start(m_loc_sb[:], m_loc.rearrange("q k p n -> p q k n"))
            m_glb_sb = one.tile([3, NQC, 256], FP16)
            nc.sync.dma_start(m_glb_sb[:], m_glb.rearrange("q g n -> g q n"))
            selT_sb = one.tile([128, NT, 3], F32)
            nc.sync.dma_start(selT_sb[:], selT.rearrange("(t p) g -> p t g", p=128))
            scat_sb = one.tile([3, T], FP16)
            nc.sync.dma_start(scat_sb[:], scat[:])
            oneh_sb = one.tile([1, T], FP16)
            nc.sync.dma_start(oneh_sb[:], oneh[:])
            wsel_sb = one.tile([128, 8], F32)
            nc.sync.dma_start(wsel_sb[:], wsel[0:1, :].to_broadcast([128, 8]))

            def ln(out_ap, in_ap, p=128):
                stats = w3.tile([p, 3, 6], F32, tag="ln_stats", name="ln_stats")
                inr = in_ap.rearrange("p (n f) -> p n f", f=256)
                for i in range(3):
                    nc.vector.bn_stats(out=stats[:, i, :], in_=inr[:, i, :])
                mv = w3.tile([p, 2], F32, tag="ln_mv", name="ln_mv")
                nc.vector.bn_aggr(out=mv[:], in_=stats[:])
                rstd = w3.tile([p, 1], F32, tag="ln_rstd", name="ln_rstd")
                nc.scalar.activation(out=rstd[:], in_=mv[:, 1:2], func=AF.Sqrt,
                                     bias=eps_t[:p])
                nc.vector.reciprocal(out=rstd[:], in_=rstd[:])
                nc.vector.tensor_scalar(out=out_ap, in0=in_ap, scalar1=mv[:, 0:1],
                                        scalar2=rstd[:], op0=ALU.subtract,
                                        op1=ALU.mult)

            # ---------------- embedding ----------------
            h = big.tile([128, NT, HID], F32, tag="h", name="h")
            xr = xin.rearrange("(t p) f -> p t f", p=128)
            er = emb.rearrange("(t p) f -> p t f", p=128)
            for tt in range(NT):
                xt = w3.tile([128, HID], F32, tag="tmp768", name="emb_x")
                et = w3.tile([128, HID], F32, tag="tmp768", name="emb_e")
                nc.sync.dma_start(xt[:], xr[:, tt])
                nc.sync.dma_start(et[:], er[:, tt])
                nc.vector.tensor_add(out=xt[:], in0=xt[:], in1=et[:])
                ln(h[:, tt, :], xt[:])

            def transpose_tm_to_fm(src_ap, dst):
                # src token-major [128, NT, 768] fp32 -> dst [128, NF, T] bf16
                for tt in range(NT):
                    cb = w3.tile([128, HID], FP16, tag="tmpb768", name="cb")
                    nc.any.tensor_copy(out=cb[:], in_=src_ap[:, tt, :])
                    for fi in range(NF):
                        pst = psT.tile([128, 512], FP16, tag="psT", name="pst")
                        nc.tensor.transpose(
                            pst[:, 0:128], cb[:, 128 * fi:128 * (fi + 1)],
                            ident_b[:])
                        nc.any.tensor_copy(
                            out=dst[:, fi, 128 * tt:128 * (tt + 1)],
                            in_=pst[:, 0:128])

            def load_w(wdram, lidx, cols=None, name="wt"):
                wt = wpool.tile([128, NF, HID], FP16, tag="wt", name=name)
                if cols is None:
                    nc.sync.dma_start(wt[:], wdram[lidx])
                else:
                    nc.sync.dma_start(wt[:], wdram[lidx, :, :, cols[0]:cols[1]])
                return wt

            def proj_fm(wt, hT, out):
                # out [128, NF, T] bf16 feature-major
                for fo in range(NF):
                    for c in range(T // 512):
                        psb = psB.tile([128, 512], F32, tag="psB", name="psb")
                        for fi in range(NF):
                            nc.tensor.matmul(
                                psb[:], wt[:, fi, 128 * fo:128 * (fo + 1)],
                                hT[:, fi, 512 * c:512 * (c + 1)],
                                start=(fi == 0), stop=(fi == NF - 1))
                        nc.any.tensor_copy(
                            out=out[:, fo, 512 * c:512 * (c + 1)], in_=psb[:])

            def proj_tm(wt, hT, out):
                # out [128, NT, 768] bf16 token-major
                for tt in range(NT):
                    for n0, nw in ((0, 512), (512, 256)):
                        psb = psB.tile([128, 512], F32, tag="psB", name="psb")
                        for fi in range(NF):
                            nc.tensor.matmul(
                                psb[:, :nw], hT[:, fi, 128 * tt:128 * (tt + 1)],
                                wt[:, fi, n0:n0 + nw],
                                start=(fi == 0), stop=(fi == NF - 1))
                        nc.any.tensor_copy(
                            out=out[:, tt, n0:n0 + nw], in_=psb[:, :nw])

            def small_proj(srcT, wt, name, odt=FP16):
                # srcT [128, NF, 3] -> [3, 768] bf16 token-major
                ps3a = psS.tile([128, 512], F32, tag="psS", name="ps3a")
                ps3b = psS.tile([128, 512], F32, tag="psS", name="ps3b")
                for pss, (n0, nw) in ((ps3a, (0, 512)), (ps3b, (512, 256))):
                    for fi in range(NF):
                        nc.tensor.matmul(pss[0:3, :nw], srcT[:, fi, :],
                                         wt[:, fi, n0:n0 + nw],
                                         start=(fi == 0), stop=(fi == NF - 1))
                ob = tiny.tile([3, HID], odt, tag=name, name=name)
                nc.any.tensor_copy(out=ob[:, 0:512], in_=ps3a[0:3, :512])
                nc.any.tensor_copy(out=ob[:, 512:768], in_=ps3b[0:3, :256])
                return ob

            def transpose3(src, name):
                # [3, 768] bf16 -> [128, NF, 3] bf16
                out = tiny.tile([128, NF, 3], FP16, tag=name + "T", name=name + "T")
                for fi in range(NF):
                    pst = psT.tile([128, 512], FP16, tag="psT", name="pst3")
                    nc.tensor.transpose(pst[:, 0:3], src[:, 128 * fi:128 * (fi + 1)],
                                        ident_b[0:3, 0:3])
                    nc.any.tensor_copy(out=out[:, fi, :], in_=pst[:, 0:3])
                return out

            hT = None
            for l in range(nlayers):
                last = (l == L - 1) and nlayers == L
                # -------- h^T --------
                hT = hpool.tile([128, NF, T], FP16, tag="hT", name="hT")
                transpose_tm_to_fm(h, hT)

                # -------- hg extraction + AG1a --------
                ps_hg1 = psS.tile([128, 512], F32, tag="psS", name="ps_hg1")
                ps_hg2 = psS.tile([128, 512], F32, tag="psS", name="ps_hg2")
                for pss, (n0, nw) in ((ps_hg1, (0, 512)), (ps_hg2, (512, 256))):
                    for tt in range(NT):
                        nc.tensor.matmul(pss[0:3, :nw], selT_sb[:, tt, :],
                                         h[:, tt, n0:n0 + nw],
                                         start=(tt == 0), stop=(tt == NT - 1))
                hg_cand = work1.tile([4, HID], F32, tag="slabx", name="hg_cand")
                nc.vector.memset(hg_cand[:], 0.0)
                nc.any.tensor_copy(out=hg_cand[0:3, 0:512], in_=ps_hg1[0:3, :512])
                nc.any.tensor_copy(out=hg_cand[0:3, 512:768], in_=ps_hg2[0:3, :256])
                ag1a_in = dram.tile([4, HID], F32, tag="ag1a_in", name="ag1a_in")
                ag1a_out = dram.tile([SP, 4, HID], F32, tag="ag1a_out",
                                     name="ag1a_out")
                nc.gpsimd.dma_start(ag1a_in[:], hg_cand[:])
                nc.gpsimd.collective_compute(
                    "AllGather", ALU.bypass, replica_groups=RG,
                    ins=[ag1a_in.opt()], outs=[ag1a_out.opt()])

                if not last:
                    # -------- k^T, v projections + AG1b --------
                    wt_k = load_w(wk, l, name="wt_k")
                    kT = hpool.tile([128, NF, T], FP16, tag="kT_mid", name="kT")
                    proj_fm(wt_k, hT, kT)
                    wt_v = load_w(wv, l, name="wt_v")
                    v_sb = hpool.tile([128, NT, HID], FP16, tag="v", name="v_sb")
                    proj_tm(wt_v, hT, v_sb)

                    ag1k_in = dram.tile([KB_ELEMS], FP16, tag="ag1k_in",
                                        name="ag1k_in")
                    ag1k_out = dram.tile([SP, KB_ELEMS], FP16, tag="ag1k_out",
                                         name="ag1k_out")
                    ag1v_in = dram.tile([VB_ELEMS], FP16, tag="ag1v_in",
                                        name="ag1v_in")
                    ag1v_out = dram.tile([SP, VB_ELEMS], FP16, tag="ag1v_out",
                                         name="ag1v_out")
                    kb = ag1k_in.rearrange("(s p f n) -> s p f n", s=2, p=128, f=NF)
                    vb = ag1v_in.rearrange("(s p n) -> s p n", s=2, p=128)
                    nc.gpsimd.dma_start(kb[0], kT[:, :, 0:128])
                    nc.gpsimd.dma_start(kb[1], kT[:, :, T - 128:T])
                    nc.gpsimd.dma_start(vb[0], v_sb[:, 0, :])
                    nc.gpsimd.dma_start(vb[1], v_sb[:, NT - 1, :])
                    nc.gpsimd.collective_compute(
                        "AllGather", ALU.bypass, replica_groups=RG,
                        ins=[ag1k_in.opt()], outs=[ag1k_out.opt()])
                    nc.gpsimd.collective_compute(
                        "AllGather", ALU.bypass, replica_groups=RG,
                        ins=[ag1v_in.opt()], outs=[ag1v_out.opt()])

                # -------- kgf^T, vgf --------
                wt_kg = load_w(wkg, l, name="wt_kg")
                kgfT = hpool.tile([128, NF, T], FP16, tag="kgfT_attnT", name="kgfT")
                proj_fm(wt_kg, hT, kgfT)
                wt_vg = load_w(wvg, l, name="wt_vg")
                vgf = hpool.tile([128, NT, HID], FP16, tag="qT_vgf", name="vgf")
                proj_tm(wt_vg, hT, vgf)

                if STAGE < 2:
                    for tt in range(NT):
                        dmy = w3.tile([128, HID], F32, tag="tmp768", name="dmy")
                        nc.vector.tensor_copy(out=dmy[:], in_=h[:, tt, :])
                        ln(h[:, tt, :], dmy[:])
                    continue
                # -------- combine hg; qg/kg/vg --------
                hg_f = work1.tile([3, HID], F32, tag="hgf", name="hg_f")
                nc.sync.dma_start(hg_f[:], ag1a_out[0, 0:3])
                for j in range(1, SP):
                    hgs = w2.tile([3, AG2_COLS], F32, tag="agld", name="hgs")
                    nc.sync.dma_start(hgs[:, 0:HID], ag1a_out[j, 0:3])
                    nc.vector.tensor_add(out=hg_f[:], in0=hg_f[:],
                                         in1=hgs[:, 0:HID])
                hg = tiny.tile([3, HID], FP16, tag="qgkg", name="hg")
                nc.vector.tensor_copy(out=hg[:], in_=hg_f[:])
                hgT = transpose3(hg, "hg")
                wt_qg = load_w(wqg, l, name="wt_qg")
                qg = small_proj(hgT, wt_qg, "qgkg")
                qgT = transpose3(qg, "qg")
                if not last:
                    wt_kg2 = load_w(wk, l, name="wt_kg2")
                    kg = small_proj(hgT, wt_kg2, "qgkg")
                    kgT = transpose3(kg, "kg")
                    wt_vg2 = load_w(wv, l, name="wt_vg2")
                    vg = small_proj(hgT, wt_vg2, "vg")

                # -------- global-row partial attention + AG2 --------
                # heads split even/odd across two psum tiles: concurrent
                # row-group matmuls must not share a PSUM bank.
                Eg2 = w2.tile([128, NT, 48], FP16, tag="Eg2", name="Eg2")
                for tt in range(NT):
                    ps_sgA = psS.tile([128, 512], F32, tag="psS", name="ps_sgA")
                    ps_sgB = psS.tile([128, 512], F32, tag="psS", name="ps_sgB")
                    nc.vector.memset(ps_sgA[:, 0:24], 0.0)
                    nc.vector.memset(ps_sgB[:, 0:24], 0.0)
                    for hh in range(NH):
                        pt = ps_sgA if hh % 2 == 0 else ps_sgB
                        nc.tensor.matmul(
                            pt[0:128, 4 * (hh // 2):4 * (hh // 2) + 3],
                            kgfT[64 * (hh % 2):64 * (hh % 2) + 64, hh // 2,
                                 128 * tt:128 * (tt + 1)],
                            qgT[64 * (hh % 2):64 * (hh % 2) + 64, hh // 2, :],
                            start=True, stop=True)
                    nc.scalar.activation(out=Eg2[:, tt, 0:24], in_=ps_sgA[:, 0:24],
                                         func=AF.Exp, bias=expb[:])
                    nc.scalar.activation(out=Eg2[:, tt, 24:48], in_=ps_sgB[:, 0:24],
                                         func=AF.Exp, bias=expb[:])
                ps_po1 = psS.tile([128, 512], F32, tag="psS", name="ps_po1")
                ps_po2 = psS.tile([128, 512], F32, tag="psS", name="ps_po2")
                ps_pdt = psB.tile([128, 512], F32, tag="psB", name="ps_pdt")
                ps_pd = ps_pdt[:, 0:24]
                nc.vector.memset(ps_pd[0:3, :], 0.0)
                for hh in range(NH):
                    po, n0 = (ps_po1, 0) if hh < 8 else (ps_po2, 512)
                    for tt in range(NT):
                        c0 = 24 * (hh % 2) + 4 * (hh // 2)
                        nc.tensor.matmul(ps_pd[0:3, 2 * hh:2 * hh + 1],
                                         Eg2[:, tt, c0:c0 + 3],
                                         ones_bf[:, :],
                                         start=(tt == 0), stop=(tt == NT - 1))
                        nc.tensor.matmul(
                            po[0:3, 64 * hh - n0:64 * hh - n0 + 64],
                            Eg2[:, tt, c0:c0 + 3],
                            vgf[:, tt, 64 * hh:64 * hh + 64],
                            start=(tt == 0), stop=(tt == NT - 1))
                ag2_in = dram.tile([3, AG2_COLS], F32, tag="ag2_in", name="ag2_in")
                ag2_out = dram.tile([SP, 3, AG2_COLS], F32, tag="ag2_out",
                                    name="ag2_out")
                slab = work1.tile([3, AG2_COLS], F32, tag="slabx", name="slab")
                nc.any.tensor_copy(out=slab[:, 0:512], in_=ps_po1[0:3, :512])
                nc.any.tensor_copy(out=slab[:, 512:768], in_=ps_po2[0:3, :256])
                nc.any.tensor_copy(out=slab[:, 768:792], in_=ps_pd[0:3, :])
                nc.vector.memset(slab[:, 792:], 0.0)
                nc.gpsimd.dma_start(ag2_in[:], slab[:])
                nc.gpsimd.collective_compute(
                    "AllGather", ALU.bypass, replica_groups=RG,
                    ins=[ag2_in.opt()], outs=[ag2_out.opt()])

                # -------- AG2 combine -> cmb --------
                psum_ = work1.tile([3, AG2_COLS], F32, tag="psumx", name="psum_")
                nc.sync.dma_start(psum_[:], ag2_out[0])
                for j in range(1, SP):
                    a2s = w2.tile([3, AG2_COLS], F32, tag="agld", name="a2s")
                    nc.sync.dma_start(a2s[:], ag2_out[j])
                    nc.vector.tensor_add(out=psum_[:], in0=psum_[:], in1=a2s[:])
                rd = tiny.tile([3, 24], F32, tag="rd", name="rd")
                nc.vector.reciprocal(out=rd[:], in_=psum_[:, 768:792])
                cmb_f = work1.tile([3, HID], F32, tag="cmb_f", name="cmb_f")
                for hh in range(NH):
                    nc.vector.tensor_scalar_mul(
                        out=cmb_f[:, 64 * hh:64 * hh + 64],
                        in0=psum_[:, 64 * hh:64 * hh + 64],
                        scalar1=rd[:, 2 * hh:2 * hh + 1])
                cmb = tiny.tile([3, HID], FP16, tag="cmb", name="cmb")
                nc.vector.tensor_copy(out=cmb[:], in_=cmb_f[:])

                if last:
                    # -------- final layer: CLS-only path --------
                    cls_o = work1.tile([1, HID], F32, tag="clsA", name="cls_o")
                    nc.vector.tensor_copy(out=cls_o[:], in_=cmb_f[0:1, :])
                    cls_b = tiny.tile([1, HID], FP16, tag="clsb", name="cls_b")
                    nc.vector.tensor_copy(out=cls_b[:], in_=cls_o[:])

                    def transpose1(src_b, nsub, name):
                        out = tiny.tile([128, nsub, 1], FP16, tag=name + "T1",
                                        name=name + "T1")
                        for fi in range(nsub):
                            pst = psT.tile([128, 512], FP16, tag="psT",
                                           name="pst1")
                            nc.tensor.transpose(
                                pst[:, 0:1], src_b[:, 128 * fi:128 * (fi + 1)],
                                ident_b[0:1, 0:1])
                            nc.any.tensor_copy(out=out[:, fi, :], in_=pst[:, 0:1])
                        return out

                    def rowmm(srcT, wdram, lidx, nsub, name, tag):
                        # [1, 768] = srcT . W  (W streamed in [*,6,768] chunks)
                        ps_a = psS.tile([128, 512], F32, tag="psS", name="ps_ra")
                        ps_b = psS.tile([128, 512], F32, tag="psS", name="ps_rb")
                        nchunk = (nsub + NF - 1) // NF
                        for ch in range(nchunk):
                            wt_c = wpool.tile([128, NF, HID], FP16, tag="wt",
                                              name="wt_row")
                            nc.sync.dma_start(
                                wt_c[:], wdram[lidx, :, NF * ch:NF * (ch + 1), :]
                                if lidx is not None else
                                wdram[:, NF * ch:NF * (ch + 1), :])
                            for pss, (n0, nw) in ((ps_a, (0, 512)),
                                                  (ps_b, (512, 256))):
                                for fi in range(NF):
                                    k = NF * ch + fi
                                    nc.tensor.matmul(
                                        pss[0:1, :nw], srcT[:, k, :],
                                        wt_c[:, fi, n0:n0 + nw],
                                        start=(k == 0), stop=(k == nsub - 1))
                        ro = work1.tile([1, HID], F32, tag=tag, name=name)
                        nc.any.tensor_copy(out=ro[:, 0:512], in_=ps_a[0:1, :512])
                        nc.any.tensor_copy(out=ro[:, 512:768], in_=ps_b[0:1, :256])
                        return ro

                    clsT = transpose1(cls_b, NF, "cls")
                    att_o = rowmm(clsT, wo, l, NF, "att_o", "clsB")
                    nc.vector.tensor_add(out=att_o[:], in0=att_o[:],
                                         in1=h[0:1, 0, :])
                    h1c = work1.tile([1, HID], F32, tag="clsA", name="h1c")
                    ln(h1c[:], att_o[:], p=1)
                    h1cb = tiny.tile([1, HID], FP16, tag="clsb", name="h1cb")
                    nc.vector.tensor_copy(out=h1cb[:], in_=h1c[:])
                    h1cT = transpose1(h1cb, NF, "h1c")
                    midc = work1.tile([1, FF], FP16, tag="slabx", name="midc")
                    for ch in range(4):
                        wt_c = wpool.tile([128, NF, HID], FP16, tag="wt",
                                          name="wt_ffn1")
                        nc.sync.dma_start(wt_c[:],
                                          wi[l, :, :, 768 * ch:768 * (ch + 1)])
                        for c2 in range(2):
                            n0 = 768 * ch + 512 * c2
                            nw = 512 if c2 == 0 else 256
                            psf = psS.tile([128, 512], F32, tag="psS", name="psf")
                            for fi in range(NF):
                                nc.tensor.matmul(
                                    psf[0:1, :nw], h1cT[:, fi, :],
                                    wt_c[:, fi, 512 * c2:512 * c2 + nw],
                                    start=(fi == 0), stop=(fi == NF - 1))
                            nc.scalar.activation(out=midc[:, n0:n0 + nw],
                                                 in_=psf[0:1, :nw], func=AF.Gelu)
                    midcT = transpose1(midc, NFM, "midc")
                    ff_o = rowmm(midcT, wff, l, NFM, "ff_o", "clsC")
                    nc.vector.tensor_add(out=ff_o[:], in0=ff_o[:], in1=h1c[:])
                    h2c = work1.tile([1, HID], F32, tag="clsB", name="h2c")
                    ln(h2c[:], ff_o[:], p=1)
                    h2cb = tiny.tile([1, HID], FP16, tag="clsb", name="h2cb")
                    nc.vector.tensor_copy(out=h2cb[:], in_=h2c[:])
                    h2cT = transpose1(h2cb, NF, "h2c")
                    pl = rowmm(h2cT, wp, None, NF, "pl", "clsA")
                    plt = work1.tile([1, HID], F32, tag="clsC", name="plt")
                    nc.scalar.activation(out=plt[:], in_=pl[:], func=AF.Tanh)
                    nc.sync.dma_start(pooled[:], plt[:])
                    break

                if STAGE < 3:
                    for tt in range(NT):
                        dmy = w3.tile([128, HID], F32, tag="tmp768", name="dmy")
                        nc.vector.tensor_copy(out=dmy[:], in_=h[:, tt, :])
                        ln(h[:, tt, :], dmy[:])
                    continue
                # -------- q^T --------
                wt_q = load_w(wq, l, name="wt_q")
                qT = hpool.tile([128, NF, T], FP16, tag="qT_vgf", name="qT")
                proj_fm(wt_q, hT, qT)

                # -------- halo select --------
                klT = work1.tile([128, NF, 128], FP16, tag="klT", name="klT")
                krT = work1.tile([128, NF, 128], FP16, tag="krT", name="krT")
                vl = work1.tile([128, HID], FP16, tag="vl", name="vl")
                vr = work1.tile([128, HID], FP16, tag="vr", name="vr")
                kbo = ag1k_out.rearrange(
                    "s (b p f n) -> s b p f n", b=2, p=128, f=NF)
                vbo = ag1v_out.rearrange(
                    "s (b p n) -> s b p n", b=2, p=128)
                for dst, src_of, side, wofs in (
                        (klT, kbo, 1, 0), (krT, kbo, 0, 4),
                        (vl, vbo, 1, 0), (vr, vbo, 0, 4)):
                    shp = [128, NF, 128] if dst in (klT, krT) else [128, HID]
                    sdt = FP16
                    for j in range(SP):
                        sl = w3.tile(shp, sdt, tag="halo_sl", name="halo_sl")
                        nc.sync.dma_start(sl[:], src_of[j, side])
                        if j == 0:
                            nc.vector.tensor_scalar_mul(
                                out=dst[:], in0=sl[:],
                                scalar1=wsel_sb[:, wofs + j:wofs + j + 1])
                        else:
                            tmp = w3.tile(shp, sdt, tag="halo_sl",
                                          name="halo_tmp")
                            nc.vector.tensor_scalar_mul(
                                out=tmp[:], in0=sl[:],
                                scalar1=wsel_sb[:, wofs + j:wofs + j + 1])
                            nc.vector.tensor_add(out=dst[:], in0=dst[:],
                                                 in1=tmp[:])

                # -------- local attention --------
                attnT = hpool.tile([128, NF, T], FP16, tag="kgfT_attnT",
                                   name="attnT")

                def k_slice(ktl, hh):
                    p0 = 64 * (hh % 2)
                    if ktl < 0:
                        return klT[p0:p0 + 64, hh // 2, :]
                    if ktl >= NT:
                        return krT[p0:p0 + 64, hh // 2, :]
                    return kT[p0:p0 + 64, hh // 2, 128 * ktl:128 * (ktl + 1)]

                def v_slice(ktl, hh):
                    if ktl < 0:
                        return vl[:, 64 * hh:64 * hh + 64]
                    if ktl >= NT:
                        return vr[:, 64 * hh:64 * hh + 64]
                    return v_sb[:, ktl, 64 * hh:64 * hh + 64]

                for qc in (1, 2, 0, 3):
                    q0 = 256 * qc
                    kts = [2 * qc - 1, 2 * qc, 2 * qc + 1, 2 * qc + 2]
                    for hp in range(NH // 2):
                        dbc_ps = psB.tile([128, 512], F32, tag="psB",
                                          name="dbc_ps")
                        ps_pv = psB.tile([128, 512], F32, tag="psB", name="ps_pv")
                        for m in range(2):
                            hh = 2 * hp + m
                            p0 = 64 * m
                            E = w2.tile([128, 4, 256], FP16, tag="E", name="E")
                            ps_den = psS.tile([128, 512], F32, tag="psS",
                                              name="ps_den")
                            for j, ktl in enumerate(kts):
                                ps_s = psB.tile([128, 512], F32, tag="psB",
                                                name="ps_s")
                                nc.tensor.matmul(
                                    ps_s[:, 0:256], k_slice(ktl, hh),
                                    qT[p0:p0 + 64, hp, q0:q0 + 256],
                                    start=True, stop=True)
                                nc.scalar.activation(out=E[:, j, :],
                                                     in_=ps_s[:, 0:256],
                                                     func=AF.Exp, bias=expb[:])
                                eng = nc.vector if (j + hh) % 2 else nc.gpsimd
                                eng.tensor_tensor(
                                    out=E[:, j, :], in0=E[:, j, :],
                                    in1=m_loc_sb[:, qc, j, :], op=ALU.mult)
                            ps_sg2 = psS.tile([128, 512], F32, tag="psS",
                                              name="ps_sg2")
                            nc.tensor.matmul(ps_sg2[0:3, 0:256],
                                             kgT[p0:p0 + 64, hp, :],
                                             qT[p0:p0 + 64, hp, q0:q0 + 256],
                                             start=True, stop=True)
                            Eg = w2.tile([3, 256], FP16, tag="Egl", name="Egl")
                            nc.scalar.activation(out=Eg[:], in_=ps_sg2[0:3, 0:256],
                                                 func=AF.Exp, bias=expb[0:3])
                            nc.vector.tensor_tensor(out=Eg[:], in0=Eg[:],
                                                    in1=m_glb_sb[:, qc, :],
                                                    op=ALU.mult)
                            for j in range(4):
                                nc.tensor.matmul(ps_den[0:1, 0:256],
                                                 ones_bf[:, :], E[:, j, :],
                                                 start=(j == 0), stop=False)
                            nc.tensor.matmul(ps_den[0:1, 0:256], ones_bf[0:3, :],
                                             Eg[:], start=False, stop=False)
                            nc.tensor.matmul(ps_den[0:1, 0:256], ones_bf[0:1, :],
                                             oneh_sb[:, q0:q0 + 256],
                                             start=False, stop=True)
                            den = w3.tile([1, 256], F32, tag="den", name="den")
                            nc.vector.reciprocal(out=den[:], in_=ps_den[0:1, 0:256])
                            nc.tensor.matmul(dbc_ps[p0:p0 + 64, 0:256],
                                             ones_f[:, 0:64], den[:],
                                             start=True, stop=True)
                            for j, ktl in enumerate(kts):
                                nc.tensor.matmul(ps_pv[p0:p0 + 64, 0:256],
                                                 v_slice(ktl, hh), E[:, j, :],
                                                 start=(j == 0), stop=False)
                            nc.tensor.matmul(ps_pv[p0:p0 + 64, 0:256],
                                             vg[:, 64 * hh:64 * hh + 64], Eg[:],
                                             start=False, stop=False)
                            nc.tensor.matmul(ps_pv[p0:p0 + 64, 0:256],
                                             cmb[:, 64 * hh:64 * hh + 64],
                                             scat_sb[:, q0:q0 + 256],
                                             start=False, stop=True)
                        dbc = w2.tile([128, 256], F32, tag="dbc", name="dbc")
                        nc.vector.tensor_copy(out=dbc[:], in_=dbc_ps[:, 0:256])
                        nc.vector.tensor_tensor(out=attnT[:, hp, q0:q0 + 256],
                                                in0=ps_pv[:, 0:256], in1=dbc[:],
                                                op=ALU.mult)

                if STAGE < 4:
                    for tt in range(NT):
                        dmy = w3.tile([128, HID], F32, tag="tmp768", name="dmy")
                        nc.vector.tensor_copy(out=dmy[:], in_=h[:, tt, :])
                        ln(h[:, tt, :], dmy[:])
                    continue
                if dbg_attn is not None and l == 0:
                    nc.sync.dma_start(dbg_attn[:], attnT[:])
                # -------- O proj + residual + LN1 --------
                wt_o = load_w(wo, l, name="wt_o")
                h1 = big.tile([128, NT, HID], F32, tag="h1", name="h1")
                for tt in range(NT):
                    hro = w3.tile([128, HID], F32, tag="tmp768", name="hro")
                    for n0, nw in ((0, 512), (512, 256)):
                        psb = psB.tile([128, 512], F32, tag="psB", name="psb")
                        for fi in range(NF):
                            nc.tensor.matmul(
                                psb[:, :nw], attnT[:, fi, 128 * tt:128 * (tt + 1)],
                                wt_o[:, fi, n0:n0 + nw],
                                start=(fi == 0), stop=(fi == NF - 1))
                        nc.vector.tensor_add(out=hro[:, n0:n0 + nw],
                                             in0=psb[:, :nw],
                                             in1=h[:, tt, n0:n0 + nw])
                    ln(h1[:, tt, :], hro[:])

                # -------- FFN --------
                h1T = hpool.tile([128, NF, T], FP16, tag="hT", name="h1T")
                transpose_tm_to_fm(h1, h1T)
                h_new = big.tile([128, NT, HID], F32, tag="h", name="h_new")
                for quarter in range(4):
                    t0 = 256 * quarter
                    midT = hpool.tile([128, NFM, 256], FP16, tag="kT_mid",
                                      name="midT")
                    for ch in range(4):
                        wt_c = wpool.tile([128, NF, HID], FP16, tag="wt",
                                          name="wt_i")
                        nc.sync.dma_start(wt_c[:],
                                          wi[l, :, :, 768 * ch:768 * (ch + 1)])
                        for fo in range(NF):
                            psb = psB.tile([128, 512], F32, tag="psB", name="psb")
                            for fi in range(NF):
                                nc.tensor.matmul(
                                    psb[:, 0:256],
                                    wt_c[:, fi, 128 * fo:128 * (fo + 1)],
                                    h1T[:, fi, t0:t0 + 256],
                                    start=(fi == 0), stop=(fi == NF - 1))
                            nc.scalar.activation(out=midT[:, NF * ch + fo, :],
                                                 in_=psb[:, 0:256], func=AF.Gelu)
                    ps_out = [
                        psB.tile([128, 512], F32, tag="psB", name="ps_out0"),
                        psB.tile([128, 512], F32, tag="psB", name="ps_out1"),
                        psS.tile([128, 512], F32, tag="psS", name="ps_out2"),
                        psS.tile([128, 512], F32, tag="psS", name="ps_out3"),
                    ]
                    for ch in range(4):
                        wt_c = wpool.tile([128, NF, HID], FP16, tag="wt",
                                          name="wt_f")
                        nc.sync.dma_start(
                            wt_c[:], wff[l, :, NF * ch:NF * (ch + 1), :])
                        for t2 in range(2):
                            for ci, (n0, nw) in enumerate(((0, 512),
                                                          (512, 256))):
                                for fi in range(NF):
                                    k = NF * ch + fi
                                    nc.tensor.matmul(
                                        ps_out[2 * t2 + ci][:, :nw],
                                        midT[:, k, 128 * t2:128 * (t2 + 1)],
                                        wt_c[:, fi, n0:n0 + nw],
                                        start=(k == 0), stop=(k == NFM - 1))
                    for t2 in range(2):
                        tt = 2 * quarter + t2
                        ffo = w3.tile([128, HID], F32, tag="tmp768",
                                      name="ffo")
                        for ci, (n0, nw) in enumerate(((0, 512), (512, 256))):
                            nc.vector.tensor_add(
                                out=ffo[:, n0:n0 + nw],
                                in0=ps_out[2 * t2 + ci][:, :nw],
                                in1=h1[:, tt, n0:n0 + nw])
                        ln(h_new[:, tt, :], ffo[:])
                h = h_new

            if want_h:
                nc.sync.dma_start(hout.rearrange("(t p) f -> p t f", p=128), h[:])
            if nlayers < L:
                zz = one.tile([1, HID], F32)
                nc.vector.memset(zz, 0.0)
                nc.sync.dma_start(pooled[:], zz[:])

    nc.compile()
    return nc


# ---------------- host side ----------------

def _host_prep(inputs):
    bf = lambda a: np.ascontiguousarray(a).astype(bfloat16)
    f16 = lambda a: np.ascontiguousarray(a).astype(np.float16)

    def tile_w(w, scale=1.0):
        w = np.asarray(w, np.float32) * scale
        Lx, fin, fout = w.shape
        return f16(w.reshape(Lx, fin // 128, 128, fout).transpose(0, 2, 1, 3))

    x = np.concatenate([
        np.broadcast_to(np.asarray(inputs["cls_tok"], np.float32)[None],
                        (B, 1, HID)),
        np.asarray(inputs["x0"], np.float32),
        np.broadcast_to(np.asarray(inputs["sep_tok"], np.float32)[None],
                        (B, 1, HID)),
        np.asarray(inputs["x1"], np.float32),
        np.broadcast_to(np.asarray(inputs["sep_tok"], np.float32)[None],
                        (B, 1, HID)),
    ], axis=1)
    qlen = S1 + 1
    tt_types = np.concatenate([np.zeros(S - qlen, np.int32),
                               np.ones(qlen, np.int32)])
    embf = (np.asarray(inputs["pos_emb"], np.float32)[np.arange(S) + 2]
            + np.asarray(inputs["type_emb"], np.float32)[tt_types])

    for k in ("bq", "bk", "bv", "bqg", "bkg", "bvg", "bo", "bi", "bff", "bp",
              "emb_ln_b", "ln1_b", "ln2_b"):
        assert not np.any(np.asarray(inputs[k])), f"nonzero bias {k} unsupported"
    for k in ("emb_ln_w", "ln1_w", "ln2_w"):
        assert np.all(np.asarray(inputs[k]) == 1.0), f"non-unit {k} unsupported"

    scale = 1.0 / np.sqrt(DH)
    weights = {
        "wq": tile_w(inputs["Wq"], scale), "wk": tile_w(inputs["Wk"]),
        "wv": tile_w(inputs["Wv"]), "wkg": tile_w(inputs["Wkg"]),
        "wvg": tile_w(inputs["Wvg"]), "wqg": tile_w(inputs["Wqg"], scale),
        "wo": tile_w(inputs["Wo"]), "wi": tile_w(inputs["Wi"]),
        "wff": tile_w(inputs["Wff"]),
        "wp": tile_w(np.asarray(inputs["Wp"])[None])[0],
    }

    per_r = []
    g = np.array(G_IDX)
    for r in range(SP):
        t_q = r * T + np.arange(T)
        m_loc = np.zeros((NQC, 4, 128, 256), np.float32)
        for qc in range(NQC):
            tq = t_q[256 * qc:256 * (qc + 1)]
            for j in range(4):
                ktl = 2 * qc - 1 + j
                tk = r * T + ktl * 128 + np.arange(128)
                valid = (np.abs(tk[:, None] - tq[None, :]) <= WIN)
                valid &= (tk[:, None] >= 0) & (tk[:, None] < S)
                valid &= ~np.isin(tk, g)[:, None]
                valid &= ~np.isin(tq, g)[None, :]
                m_loc[qc, j] = valid
        m_glb = np.broadcast_to(~np.isin(t_q, g), (3, T)).reshape(
            3, NQC, 256).transpose(1, 0, 2).astype(np.float32)
        oneh = np.isin(t_q, g).astype(np.float32)[None]
        selTm = np.zeros((T, 3), np.float32)
        scatm = np.zeros((3, T), np.float32)
        for gi, gp in enumerate(G_IDX):
            if r * T <= gp < (r + 1) * T:
                selTm[gp - r * T, gi] = 1.0
                scatm[gi, gp - r * T] = 1.0
        wselm = np.zeros((1, 8), np.float32)
        if r > 0:
            wselm[0, r - 1] = 1.0
        if r < SP - 1:
            wselm[0, 4 + r + 1] = 1.0
        per_r.append(dict(m_loc=f16(m_loc), m_glb=f16(np.ascontiguousarray(m_glb)),
                          oneh=f16(oneh), selT=selTm, scat=f16(scatm), wsel=wselm))

    in_maps = []
    for c in range(8):
        b, r = c // SP, c % SP
        m = dict(weights)
        m.update(per_r[r])
        m["xin"] = np.ascontiguousarray(x[b, r * T:(r + 1) * T])
        m["emb"] = np.ascontiguousarray(embf[r * T:(r + 1) * T])
        in_maps.append(m)
    return in_maps


_CACHE = {}


def _get_program(nlayers=6, dbg=False):
    key = (nlayers, dbg)
    if key not in _CACHE:
        _CACHE[key] = build(nlayers, dbg)
    return _CACHE[key]


def kernel(**inputs):
    nlayers = int(os.environ.get("KERN_NLAYERS", "6"))
    dbg = bool(int(os.environ.get("KERN_DBG", "0")))
    nc = _get_program(nlayers, dbg)
    in_maps = _host_prep(inputs)
    trace = bool(int(os.environ.get("KERN_TRACE", "0")))
    res = run_bass_kernel_spmd(nc, in_maps, core_ids=list(range(8)), trace=trace)
    kernel.last_results = res
    out = np.stack([res.results[0]["pooled"], res.results[4]["pooled"]])
    return out.astype(np.float32)
